# revision 17
# baseline (speedup 1.0000x reference)
"""Trainium2 Bass kernel for nn_Attn_Module (B=8, C=512, L=2048, CP=64).

Data-parallel over batch: each of the 8 NeuronCores computes one batch element's
full attention. No collectives.

Per-core math (b = batch element):
  v  = value_w @ x[b]                  [64, 2048]
  u  = Mq^T v,  Mq = (Qw/32)^T Kw      [64, 2048]  (E/32 = u^T v; k-pass eliminated)
  row bound b_l = -max_m (u^T v)[l, m] (stats pass, DVE reduces)
  E^T_biased[j, l] = [v; 1]^T [u; b]   (bias rides the matmul as a 65th K-row)
  P^T = exp(32 * E^T_biased)           bf16, directly in AV-ready [j, l] layout
  O65 = vT65^T @ P^T accumulated over j-tiles; vT65 = [gamma*v^T | ones-col]
        rows 0-63 = gamma*out_unnorm, row 64 = S2 (softmax denominator)
  out[0:64]  = O65[0:64] / S2 ;  out[64:128] = v
"""
import sys
import types

sys.path.insert(0, '/opt/trn_rl_repo')
sys.path.insert(0, '/root/.axon_site')

import numpy as np


def _install_ntff_hook():
    try:
        import antenv
    except ImportError:
        return
    if 'antenv.axon_hooks' in sys.modules:
        return
    mod = types.ModuleType('antenv.axon_hooks')
    mod._hook = None
    mod.set_axon_ntff_profile_hook = lambda h: setattr(mod, '_hook', h)
    mod.get_axon_ntff_profile_hook = lambda: mod._hook
    sys.modules['antenv.axon_hooks'] = mod
    antenv.axon_hooks = mod
    try:
        from trn_agent_boot.trn_boot import _ntff_profile_via_ctypes
        mod.set_axon_ntff_profile_hook(_ntff_profile_via_ctypes('/opt/axon/libaxon_pjrt.so'))
    except Exception:
        pass


_install_ntff_hook()

import concourse.bacc as bacc
import concourse.mybir as mybir
from concourse.bass_utils import run_bass_kernel_spmd
from concourse.tile import TileContext

F32 = mybir.dt.float32
F32R = mybir.dt.float32r
BF16 = mybir.dt.bfloat16

B, C, L, CP = 8, 512, 2048, 64
NLT = L // 128     # 16 l-tiles
NJT = L // 128     # 16 j-tiles
NLC = L // 512     # 4 chunks
NG = NJT // 2      # 8 j-groups of 2 tiles (one [128,1024] PSUM pair each)
SCALE = 32.0
N_WARMUP = 14


def f32r_round(a):
    """Round fp32 array to the float32r grid (RNE on low 12 mantissa bits, sign-magnitude)."""
    a = np.ascontiguousarray(a, np.float32)
    xi = a.view(np.int32)
    sign = xi & np.int32(-2**31)
    mag = (xi & np.int32(0x7FFFFFFF)).astype(np.int64)
    add = 1 << 11
    mr = mag + add
    ties = (mag & ((1 << 12) - 1)) == add
    mr = np.where(ties & (((mag >> 12) & 1) == 0), mag, mr)
    mr &= ~((1 << 12) - 1)
    return (sign | mr.astype(np.int32)).view(np.float32).reshape(a.shape)


def build_nc(gamma: float, debug: bool = False):
    nc = bacc.Bacc()
    x_p = nc.declare_dram_parameter('x', [C, L], F32R, isOutput=False)
    vwT_p = nc.declare_dram_parameter('vwT', [C, CP], F32R, isOutput=False)
    mq_p = nc.declare_dram_parameter('mq', [CP, CP], F32R, isOutput=False)
    id_p = nc.declare_dram_parameter('ident', [128, 128], F32R, isOutput=False)
    out_p = nc.declare_dram_parameter('out', [128, L], F32, isOutput=True)
    if debug:
        dbg_q_p = nc.declare_dram_parameter('dbg_q', [65, L], F32, isOutput=True)
        dbg_k_p = nc.declare_dram_parameter('dbg_k', [65, L], F32, isOutput=True)
        dbg_sh_p = nc.declare_dram_parameter('dbg_sh', [128, 3 * NLT], F32, isOutput=True)
        dbg_ns_p = nc.declare_dram_parameter('dbg_ns', [128, 32], F32, isOutput=True)

    with TileContext(nc) as tc:
        with tc.tile_pool(name='sb', bufs=1) as sb, \
             tc.tile_pool(name='pt', bufs=3) as ptp, \
             tc.tile_pool(name='so', bufs=2) as sop, \
             tc.tile_pool(name='nr', bufs=4) as nrp, \
             tc.tile_pool(name='wk', bufs=3, space='PSUM') as wkp, \
             tc.tile_pool(name='oo', bufs=2, space='PSUM') as oop:

            # ---------- warm tiles (memset, no DMA dependency) ----------
            wz = sb.tile([128, 640], F32R, tag='wz')
            nc.gpsimd.memset(wz[:].bitcast(F32), 0.0)

            def emit_warm(n, name):
                wt = wkp.tile([128, 512], F32, tag='wk', name=f'warm_{name}')
                for _ in range(n):
                    nc.tensor.matmul(wt[:], wz[:, 0:128], wz[:, 128:640],
                                     start=True, stop=True)

            emit_warm(N_WARMUP, 'boot')

            # ---------- small loads first (weights before x) ----------
            ident = sb.tile([128, 128], F32R, tag='ident')
            nc.sync.dma_start(ident[:], id_p[:])
            vw = sb.tile([128, 4 * CP], F32R, tag='vw')
            for kt in range(4):
                nc.sync.dma_start(vw[:, kt * CP:(kt + 1) * CP], vwT_p[kt * 128:(kt + 1) * 128, :])
            mq = sb.tile([64, CP], F32R, tag='mq')
            nc.sync.dma_start(mq[:], mq_p[:])
            actwarm = sb.tile([1, 8], F32, tag='actwarm')
            nc.scalar.activation(actwarm[:], ident[0:1, 0:8].bitcast(F32),
                                 mybir.ActivationFunctionType.Exp, bias=0.0, scale=0.0)


            # ---------- x load: 8 tiles [128, 1024], each as 2 partition-half DMAs ----------
            xc = [[sb.tile([128, 1024], F32R, tag=f'x{kt}_{lh}', name=f'x{kt}_{lh}')
                   for lh in range(2)] for kt in range(4)]
            for lh in range(2):
                for kt in range(4):
                    for ph in range(4):
                        nc.sync.dma_start(
                            xc[kt][lh][ph * 32:(ph + 1) * 32, :],
                            x_p[kt * 128 + ph * 32:kt * 128 + (ph + 1) * 32,
                                lh * 1024:(lh + 1) * 1024])

            # ---------- persistent SBUF ----------
            K65 = sb.tile([65, L], F32R, tag='K65')   # rows 0:64 = v, row 64 = ones
            Q65 = sb.tile([65, L], F32R, tag='Q65')   # rows 0:64 = u, row 64 = brow
            nc.gpsimd.memset(K65[64:65, :].bitcast(F32), 1.0)
            vt65 = sb.tile([128, NJT * 65], BF16, tag='vt65')
            ones_col = vt65[:].rearrange('p (a b) -> p a b', b=65)[:, :, 64:65]
            nc.gpsimd.memset(ones_col, 1.0)
            statsH = sb.tile([128, 3 * NLT], F32, tag='statsH')   # per l-tile: up to 3 partial maxes
            bias45 = sb.tile([128, 3], F32, tag='bias45')   # col0 = -45, col2 = log affine const
            nc.gpsimd.memset(bias45[:, 0:1], -45.0)
            nc.gpsimd.memset(bias45[:, 2:3], float(45.0 - 1.75 - 127.0 * np.log(2.0)))
            nc.gpsimd.memset(statsH[:], -3.0e38)
            negst = sb.tile([128, 32], F32, tag='negst')          # col lt = -max (padded to 32)
            statsT = sb.tile([128, 32], F32, tag='statsT')        # 32x32 block transpose of negst

            # ---------- v / u per chunk ----------
            def emit_v(lc):
                pv = oop.tile([64, 512], F32, tag='oo', name=f'pv{lc}')
                lh, c2 = lc // 2, lc % 2
                for kt in range(4):
                    nc.tensor.matmul(pv[:], vw[:, kt * CP:(kt + 1) * CP],
                                     xc[kt][lh][:, c2 * 512:(c2 + 1) * 512],
                                     start=(kt == 0), stop=(kt == 3))
                nc.scalar.copy(K65[0:64, lc * 512:(lc + 1) * 512], pv[:])

            def emit_u(lc):
                pu = oop.tile([64, 512], F32, tag='oo', name=f'pu{lc}')
                nc.tensor.matmul(pu[:], mq[:], K65[0:64, lc * 512:(lc + 1) * 512],
                                 start=True, stop=True)
                nc.scalar.copy(Q65[0:64, lc * 512:(lc + 1) * 512], pu[:])

            # ---------- vT65 (gamma * v^T | ones col), bf16; group g covers jt 8g..8g+7 ----------
            def emit_T(g):
                pvt = oop.tile([128, 512], F32R, tag='oo', name=f'pvt{g}')
                for bi in range(8):
                    jt = g * 8 + bi
                    nc.tensor.transpose(pvt[:, bi * 64:(bi + 1) * 64],
                                        K65[0:64, jt * 128:(jt + 1) * 128],
                                        ident[0:64, 0:64])
                dst = vt65[:, g * 8 * 65:].rearrange('p (a b) -> p a b', b=65)[:, 0:8, 0:64]
                nc.scalar.mul(dst, pvt[:].rearrange('p (a b) -> p a b', b=64), float(gamma))

            # ---------- stats units: per l-tile, h0 = max over m[0:1024], h1 = m[1024:2048] ----------
            def stats_mm(lt, h):
                sg = wkp.tile([128, 1024], F32, tag='wk', name=f's{h}_{lt}')
                for i in range(2):
                    mc = 2 * h + i
                    nc.tensor.matmul(sg[:, i * 512:(i + 1) * 512],
                                     Q65[0:64, lt * 128:(lt + 1) * 128],
                                     K65[0:64, mc * 512:(mc + 1) * 512],
                                     start=True, stop=True)
                return sg

            def stats_red(lt, h, sg):
                # one [128,1024] DVE reduce -> statsH col 3lt+h
                nc.vector.reduce_max(statsH[:, 3 * lt + h:3 * lt + h + 1], sg[:],
                                     axis=mybir.AxisListType.X)

            LOG_A = float(np.log(2.0) / (1 << 23))
            LOG_B = float(45.0 - 1.75 - 127.0 * np.log(2.0))

            def stats_red_lse(lt, h, sg):
                # ACT-side stats: acc = sum exp(E'-45) on the scalar engine
                # (exp table set only — no swaps); then ln via the float-bits
                # log2 approximation on DVE:
                #   proxy = ln(acc) + 45 - 1.75
                #         ~ bits(acc)*ln2/2^23 + (43.25 - 127*ln2)   (err <= 0.06)
                sof = sop.tile([128, 1024], BF16, tag='so', name=f'so{lt}_{h}')
                acc = sop.tile([128, 1], F32, tag='soa', name=f'soa{lt}_{h}')
                nc.scalar.activation(sof[:], sg[:], mybir.ActivationFunctionType.Exp,
                                     bias=bias45[:, 0:1], scale=1.0, accum_out=acc[:])
                tmp = sop.tile([128, 1], F32, tag='sot', name=f'sot{lt}_{h}')
                nc.vector.tensor_copy(tmp[:], acc[:].bitcast(mybir.dt.int32))
                nc.vector.scalar_tensor_tensor(
                    statsH[:, 3 * lt + h:3 * lt + h + 1], tmp[:], LOG_A,
                    bias45[:, 2:3], op0=mybir.AluOpType.mult, op1=mybir.AluOpType.add)

            def stats_red_split(lt, h, sg):
                # two single-bank reduces (lower latency; cols 3lt+1, 3lt+2)
                for i in range(2):
                    nc.vector.reduce_max(statsH[:, 3 * lt + 1 + i:3 * lt + 2 + i],
                                         sg[:, i * 512:(i + 1) * 512],
                                         axis=mybir.AxisListType.X)


            def emit_brow(lc):
                # combine the 3 partial maxes -> negst cols [4lc:4lc+4], negated
                src = statsH[:, 12 * lc:12 * lc + 12].rearrange('p (lt s) -> p lt s', s=3)
                nc.vector.reduce_max(negst[:, 4 * lc:4 * lc + 4], src,
                                     axis=mybir.AxisListType.X, negate=True)
                # 32x32 block transpose: negst[32a+j, lt] -> statsT[32a+lt, j]
                nc.vector.transpose(statsT[:], negst[:])
                # statsT[32a + lt, j] -> Q65 row 64, col lt*128 + 32a + j  (lt global)
                dst_all = Q65[64:65, lc * 512:(lc + 1) * 512].bitcast(F32).rearrange(
                    'p (i a j) -> p i a j', i=4, a=4)
                for a in range(4):
                    src_ap = statsT[32 * a + 4 * lc:32 * a + 4 * lc + 4, :].rearrange(
                        'q (z j) -> q z j', z=1)
                    nc.sync.dma_start(dst_all[:, :, a:a + 1, :], src_ap)

            # ---------- prologue ----------
            emit_warm(16, 'pre_v0')
            emit_v(0)
            emit_u(0)
            emit_warm(8, 'f01')
            emit_v(1)
            emit_u(1)
            emit_T(0)
            emit_warm(8, 'f02')
            for lt in range(4):
                sg = stats_mm(lt, 0)
                stats_red(lt, 0, sg)
                emit_warm(2, f'sA{lt}')
            emit_warm(8, 'f03')
            emit_v(2)
            emit_u(2)
            emit_warm(8, 'f04')
            emit_v(3)
            emit_u(3)
            emit_T(1)
            emit_warm(4, 'f05')
            for lt in range(4):
                sg = stats_mm(lt, 1)
                if lt % 2 == 0:
                    stats_red_split(lt, 1, sg)
                else:
                    stats_red_lse(lt, 1, sg)
                emit_warm(3, f'sB{lt}')
            emit_warm(10, 'pre_body')
            emit_brow(0)
            # v output channels (final): overlap store with the body
            nc.scalar.dma_start(out_p[64:128, :], K65[0:64, :].bitcast(F32))

            # ---------- body ----------
            o65 = [None] * NLC
            pts = [None] * NG

            def emit_et_group(lc, g):
                eg = wkp.tile([128, 1024], F32, tag='wk', name=f'e{lc}_{g}')
                for i in range(2):
                    jt = 2 * g + i
                    nc.tensor.matmul(eg[:, i * 512:(i + 1) * 512],
                                     K65[0:65, jt * 128:(jt + 1) * 128],
                                     Q65[0:65, lc * 512:(lc + 1) * 512],
                                     start=True, stop=True)
                pg = ptp.tile([128, 1024], BF16, tag='pt', name=f'p{lc}_{g}')
                nc.scalar.activation(pg[:], eg[:], mybir.ActivationFunctionType.Exp,
                                     bias=0.0, scale=SCALE)
                pts[g] = pg

            def emit_av(lc, g):
                for i in range(2):
                    jt = 2 * g + i
                    nc.tensor.matmul(o65[lc][:], vt65[:, jt * 65:(jt + 1) * 65],
                                     pts[g][:, i * 512:(i + 1) * 512],
                                     start=(jt == 0), stop=(jt == NJT - 1))

            def emit_norm(lc):
                s2s = nrp.tile([1, 512], F32, tag='nr', name=f's2_{lc}')
                nc.scalar.copy(s2s[:], o65[lc][64:65, :])
                r1 = nrp.tile([1, 512], F32, tag='nr', name=f'r1_{lc}')
                nc.vector.reciprocal_approx_fast(r1[:], s2s[:])
                r2 = nrp.tile([64, 512], F32, tag='nr', name=f'r2_{lc}')
                nc.gpsimd.partition_broadcast(r2[:], r1[:])
                ofin = nrp.tile([64, 512], F32, tag='nr', name=f'of{lc}')
                nc.vector.tensor_tensor(ofin[:], o65[lc][0:64, :], r2[:],
                                        op=mybir.AluOpType.mult)
                nc.sync.dma_start(out_p[0:64, lc * 512:(lc + 1) * 512], ofin[:])

            for lc in range(NLC):
                emit_warm(13 if lc == 0 else 3, f'it{lc}')
                o65[lc] = oop.tile([65, 512], F32, tag='oo', name=f'o65_{lc}')
                nlc = lc + 1
                # stats micro-step schedule for chunk nlc: per lt:
                #   [mm h0][red h0][mm h1][red h1(offload 1 of 4)]
                steps = []
                if nlc < NLC:
                    sgs = {}
                    for i in range(4):
                        lt = 4 * nlc + i
                        steps.append(lambda lt=lt: sgs.__setitem__(lt, stats_mm(lt, 0)))
                        steps.append(lambda lt=lt: stats_red(lt, 0, sgs[lt]))
                        steps.append(lambda lt=lt: sgs.__setitem__(lt, stats_mm(lt, 1)))
                        if i == 1:
                            steps.append(lambda lt=lt: stats_red_lse(lt, 1, sgs[lt]))
                        else:
                            steps.append(lambda lt=lt: stats_red(lt, 1, sgs[lt]))
                si = 0
                npre = 4 if lc == 0 else 1
                while si < min(npre, len(steps)):
                    steps[si]()
                    si += 1
                for g in range(NG):
                    emit_et_group(lc, g)
                    for _ in range(2):
                        if si < len(steps):
                            steps[si]()
                            si += 1
                    if g > 0:
                        emit_av(lc, g - 1)
                    if g % 2 == 0:
                        emit_warm(1, f'f{lc}_{g}')
                while si < len(steps):
                    steps[si]()
                    si += 1
                emit_av(lc, NG - 1)
                if nlc < NLC:
                    emit_brow(nlc)
                emit_norm(lc)

            if debug:
                nc.sync.dma_start(dbg_q_p[:], Q65[:].bitcast(F32))
                nc.sync.dma_start(dbg_k_p[:], K65[:].bitcast(F32))
                nc.sync.dma_start(dbg_sh_p[:], statsH[:])
                nc.sync.dma_start(dbg_ns_p[:], negst[:])

    nc.finalize()
    return nc


_cache = {}


def _get_nc(gamma: float):
    key = float(gamma)
    if key not in _cache:
        _cache[key] = build_nc(key)
    return _cache[key]


def _in_maps(inputs):
    x = np.asarray(inputs['x'], np.float32)
    vwT = f32r_round(np.asarray(inputs['value_w'], np.float32).T)
    qw = np.asarray(inputs['query_w'], np.float32)
    kw = np.asarray(inputs['key_w'], np.float32)
    mq = f32r_round((qw.T / SCALE) @ kw)
    ident = np.eye(128, dtype=np.float32)
    xs = f32r_round(x[..., 0])
    return [
        {'x': np.ascontiguousarray(xs[b]), 'vwT': vwT, 'mq': mq, 'ident': ident}
        for b in range(B)
    ]


def kernel(x, value_w, value_b, query_w, query_b, key_w, key_b, gamma):
    gamma_f = float(np.asarray(gamma).reshape(-1)[0])
    nc = _get_nc(gamma_f)
    maps = _in_maps(dict(x=x, value_w=value_w, query_w=query_w, key_w=key_w))
    res = run_bass_kernel_spmd(nc, maps, core_ids=list(range(B)), trace=False)
    out = np.stack([res.results[b]['out'] for b in range(B)], axis=0)
    return out[..., None].astype(np.float32)


def run_traced(inputs):
    gamma_f = float(np.asarray(inputs['gamma']).reshape(-1)[0])
    nc = _get_nc(gamma_f)
    maps = _in_maps(inputs)
    res = run_bass_kernel_spmd(nc, maps, core_ids=list(range(B)), trace=True)
    out = np.stack([res.results[b]['out'] for b in range(B)], axis=0)
    return out[..., None].astype(np.float32), res.exec_time_ns


# revision 19
# speedup vs baseline: 1.1920x; 1.1920x over previous
"""Trainium2 Bass kernel for nn_Attn_Module (B=8, C=512, L=2048, CP=64).

Data-parallel over batch: each of the 8 NeuronCores computes one batch element's
full attention. No collectives.

Per-core math (b = batch element):
  v  = value_w @ x[b]                [64, 2048]
  q' = (query_w/32) @ v              [64, 2048]   (1/32 logit scale folded into weights)
  k  = key_w @ v                     [64, 2048]
  E' = q'^T k = E/32                 (computed per l-tile for row stats only)
  row bound b_l = -max_m E'[l, m]    (DVE tiles) or -(ln sum exp E' - 2) (ACT tiles)
  E^T_biased[j, l] = k^T q' + ones*b_row   (bias rides the matmul as a 65th K-row)
  P^T = exp(32 * E^T_biased)         bf16, directly in AV-ready [j, l] layout
  O65 = vT65^T @ P^T accumulated over j-tiles; vT65 = [gamma*v^T | ones-col]
        rows 0-63 = gamma*out_unnorm, row 64 = S2 (softmax denominator)
  out[0:64]  = O65[0:64] / S2 ;  out[64:128] = v
"""
import sys
import types

sys.path.insert(0, '/opt/trn_rl_repo')
sys.path.insert(0, '/root/.axon_site')

import numpy as np


def _install_ntff_hook():
    try:
        import antenv
    except ImportError:
        return
    if 'antenv.axon_hooks' in sys.modules:
        return
    mod = types.ModuleType('antenv.axon_hooks')
    mod._hook = None
    mod.set_axon_ntff_profile_hook = lambda h: setattr(mod, '_hook', h)
    mod.get_axon_ntff_profile_hook = lambda: mod._hook
    sys.modules['antenv.axon_hooks'] = mod
    antenv.axon_hooks = mod
    try:
        from trn_agent_boot.trn_boot import _ntff_profile_via_ctypes
        mod.set_axon_ntff_profile_hook(_ntff_profile_via_ctypes('/opt/axon/libaxon_pjrt.so'))
    except Exception:
        pass


_install_ntff_hook()

import concourse.bacc as bacc
import concourse.mybir as mybir
from concourse.bass_utils import run_bass_kernel_spmd
from concourse.tile import TileContext

F32 = mybir.dt.float32
F32R = mybir.dt.float32r
BF16 = mybir.dt.bfloat16

B, C, L, CP = 8, 512, 2048, 64
NLT = L // 128
NJT = L // 128
NLC = L // 512
SCALE = 32.0

# stat style per l-tile PAIR (8 pairs): True = ACT LSE, False = DVE max
PAIR_ON_ACT = [False] * 8


def f32r_round(a):
    """Round fp32 array to the float32r grid (RNE on low 12 mantissa bits, sign-magnitude)."""
    a = np.ascontiguousarray(a, np.float32)
    xi = a.view(np.int32)
    sign = xi & np.int32(-2**31)
    mag = (xi & np.int32(0x7FFFFFFF)).astype(np.int64)
    add = 1 << 11
    mr = mag + add
    ties = (mag & ((1 << 12) - 1)) == add
    mr = np.where(ties & (((mag >> 12) & 1) == 0), mag, mr)
    mr &= ~((1 << 12) - 1)
    return (sign | mr.astype(np.int32)).view(np.float32).reshape(a.shape)


def build_nc(gamma: float):
    nc = bacc.Bacc()
    x_p = nc.declare_dram_parameter('x', [C, L], F32R, isOutput=False)
    vwT_p = nc.declare_dram_parameter('vwT', [C, CP], F32R, isOutput=False)
    qwT_p = nc.declare_dram_parameter('qwT', [CP, CP], F32R, isOutput=False)
    kwT_p = nc.declare_dram_parameter('kwT', [CP, CP], F32R, isOutput=False)
    id_p = nc.declare_dram_parameter('ident', [128, 128], F32R, isOutput=False)
    out_p = nc.declare_dram_parameter('out', [128, L], F32, isOutput=True)

    LNC = 2.0 - 24 * float(np.log(2.0))

    with TileContext(nc) as tc:
        with tc.tile_pool(name='sb', bufs=1) as sb, \
             tc.tile_pool(name='pt', bufs=8) as ptp, \
             tc.tile_pool(name='scr', bufs=6) as scr, \
             tc.tile_pool(name='wk', bufs=6, space='PSUM') as wkp, \
             tc.tile_pool(name='oo', bufs=2, space='PSUM') as oo:

            # ---------- loads: weights first, then x in lc-major quarter-tiles ----------
            ident = sb.tile([128, 128], F32R, tag='ident')
            nc.sync.dma_start(ident[:], id_p[:])
            vw = sb.tile([128, 4 * CP], F32R, tag='vw')
            for kt in range(4):
                nc.sync.dma_start(vw[:, kt * CP:(kt + 1) * CP], vwT_p[kt * 128:(kt + 1) * 128, :])
            qkw = sb.tile([64, 2 * CP], F32R, tag='qkw')
            nc.sync.dma_start(qkw[:, 0:CP], qwT_p[:])
            nc.sync.dma_start(qkw[:, CP:2 * CP], kwT_p[:])
            # PE warmup: same-tile matmuls on memset data (no DMA dependency) to
            # open the HAM clock gate before real work arrives
            wz = sb.tile([128, 640], F32R, tag='wz')
            nc.gpsimd.memset(wz[:].bitcast(F32), 0.0)
            wtile = wkp.tile([128, 512], F32, tag='wk', name='warmt')
            for w in range(14):
                nc.tensor.matmul(wtile[:], wz[:, 0:128], wz[:, 128:640],
                                 start=True, stop=True)
            xc = [[sb.tile([128, 512], F32R, tag=f'x{kt}_{lc}', name=f'x{kt}_{lc}')
                   for lc in range(NLC)] for kt in range(4)]
            for lc in range(NLC):
                for kt in range(4):
                    for ph in range(2):
                        nc.sync.dma_start(
                            xc[kt][lc][ph * 64:(ph + 1) * 64, :],
                            x_p[kt * 128 + ph * 64:kt * 128 + (ph + 1) * 64,
                                lc * 512:(lc + 1) * 512])

            # ---------- v = value_w @ x (chunked, dup DMAs per chunk) ----------
            v_sb = sb.tile([64, L], F32R, tag='v')
            for lc in range(NLC):
                pv = wkp.tile([64, 512], F32, tag='wk', name=f'pv{lc}')
                for kt in range(4):
                    nc.tensor.matmul(pv[:], vw[:, kt * CP:(kt + 1) * CP],
                                     xc[kt][lc][:],
                                     start=(kt == 0), stop=(kt == 3))
                nc.scalar.copy(v_sb[:, lc * 512:(lc + 1) * 512], pv[:])

            # ---------- vT65 (v^T * gamma | ones col), bf16 ----------
            vt65 = sb.tile([128, NJT * 65], BF16, tag='vt65')
            for g in range(2):
                pvt = wkp.tile([128, 512], F32R, tag='wk', name=f'pvt{g}')
                for bi in range(8):
                    jt = g * 8 + bi
                    nc.tensor.transpose(pvt[:, bi * 64:(bi + 1) * 64],
                                        v_sb[:, jt * 128:(jt + 1) * 128],
                                        ident[0:64, 0:64])
                dst = vt65[:, g * 8 * 65:].rearrange('p (a b) -> p a b', b=65)[:, 0:8, 0:64]
                nc.scalar.mul(dst, pvt[:].rearrange('p (a b) -> p a b', b=64), float(gamma))
            ones_col = vt65[:].rearrange('p (a b) -> p a b', b=65)[:, :, 64:65]
            nc.gpsimd.memset(ones_col, 1.0)

            # ---------- q', k and Q65/K65 ----------
            q_sb = sb.tile([64, L], F32R, tag='q_sb')
            k_sb = sb.tile([64, L], F32R, tag='k_sb')
            Q65 = sb.tile([128, L], F32R, tag='Q65')   # row 0 = bias row (written later)
            K65 = sb.tile([128, L], F32R, tag='K65')   # row 0 = ones
            nc.gpsimd.memset(K65[0:1, :].bitcast(F32), 1.0)
            for lc in range(NLC):
                sl = slice(lc * 512, (lc + 1) * 512)
                pq = wkp.tile([64, 512], F32, tag='wk', name=f'pq{lc}')
                pk = wkp.tile([64, 512], F32, tag='wk', name=f'pk{lc}')
                nc.tensor.matmul(pq[:], qkw[:, 0:CP], v_sb[:, sl], start=True, stop=True)
                nc.tensor.matmul(pk[:], qkw[:, CP:2 * CP], v_sb[:, sl], start=True, stop=True)
                nc.scalar.copy(q_sb[:, sl], pq[:])
                nc.scalar.copy(k_sb[:, sl], pk[:])
                e1 = nc.sync if lc % 2 == 0 else nc.scalar
                e2 = nc.scalar if lc % 2 == 0 else nc.sync
                e1.dma_start(Q65[1:65, sl], q_sb[:, sl])
                e2.dma_start(K65[1:65, sl], k_sb[:, sl])

            # v output channels: v_sb is final here; overlap the store with stats
            nc.sync.dma_start(out_p[64:128, :], v_sb[:].bitcast(F32))

            stats = sb.tile([128, NLT], F32R, tag='stats')

            # ---------- stats: one (pair, mc) step; 8 steps per l-chunk ----------
            def stat_steps(lc):
                """Return a list of closures; each emits one stat matmul-pair + reduce."""
                steps = []
                for half in range(2):
                    pair = lc * 2 + half
                    ltA, ltB = 2 * pair, 2 * pair + 1
                    mx = scr.tile([128, 8], F32, tag='mx', name=f'mx{pair}')

                    def mk_mc(pair, ltA, ltB, mx, mc, last):
                        def step():
                            ppA = wkp.tile([128, 512], F32, tag='wk', name=f'ppA{pair}_{mc}')
                            ppB = wkp.tile([128, 512], F32, tag='wk', name=f'ppB{pair}_{mc}')
                            nc.tensor.matmul(ppA[:], q_sb[:, ltA * 128:(ltA + 1) * 128],
                                             k_sb[:, mc * 512:(mc + 1) * 512], start=True, stop=True)
                            nc.tensor.matmul(ppB[:], q_sb[:, ltB * 128:(ltB + 1) * 128],
                                             k_sb[:, mc * 512:(mc + 1) * 512], start=True, stop=True)
                            nc.vector.reduce_max(mx[:, 2 * mc:2 * mc + 1], ppA[:],
                                                 axis=mybir.AxisListType.X)
                            nc.vector.reduce_max(mx[:, 2 * mc + 1:2 * mc + 2], ppB[:],
                                                 axis=mybir.AxisListType.X)
                            if last:
                                for i, lt in ((0, ltA), (1, ltB)):
                                    sub = mx[:].rearrange('p (a b) -> p a b', b=2)[:, :, i:i + 1]
                                    nc.vector.reduce_max(stats[:, lt:lt + 1], sub,
                                                         axis=mybir.AxisListType.XY, negate=True)
                        return step
                    for mc in range(NLC):
                        steps.append(mk_mc(pair, ltA, ltB, mx, mc, mc == NLC - 1))
                return steps

            def emit_brow(lc):
                pb = wkp.tile([4, 128], F32R, tag='wk', name=f'pb{lc}')
                nc.tensor.transpose(pb[:], stats[:, lc * 4:(lc + 1) * 4], ident[:])
                bs = scr.tile([4, 128], F32R, tag='bs', name=f'bs{lc}')
                nc.vector.tensor_copy(bs[:], pb[:])
                nc.sync.dma_start(
                    Q65[0:1, lc * 512:(lc + 1) * 512].rearrange('p (a b) -> p a b', b=128),
                    bs[:])

            o65 = [oo.tile([65, 512], F32, tag='oo', name=f'o65_{lc}') for lc in range(NLC)]

            def emit_stats(lc):
                for st in stat_steps(lc):
                    st()

            emit_stats(0)
            emit_brow(0)
            emit_stats(1)

            def emit_norm(lc):
                r1 = scr.tile([1, 512], F32, tag='r1', name=f'r1_{lc}')
                s2 = scr.tile([1, 512], F32, tag='s2', name=f's2_{lc}')
                nc.vector.tensor_copy(s2[:], o65[lc][64:65, :])
                nc.vector.reciprocal_approx_fast(r1[:], s2[:])
                r2 = scr.tile([64, 512], F32, tag='r2', name=f'r2_{lc}')
                nc.gpsimd.partition_broadcast(r2[:], r1[:])
                ofin = scr.tile([64, 512], F32, tag='ofin', name=f'of{lc}')
                nc.vector.tensor_tensor(ofin[:], o65[lc][0:64, :], r2[:], op=mybir.AluOpType.mult)
                nc.sync.dma_start(out_p[0:64, lc * 512:(lc + 1) * 512], ofin[:])

            HOIST = 4

            def emit_cunit(lc, jt, pts):
                e = wkp.tile([128, 512], F32, tag='wk', name=f'e{lc}_{jt}')
                nc.tensor.matmul(e[:], K65[0:65, jt * 128:(jt + 1) * 128],
                                 Q65[0:65, lc * 512:(lc + 1) * 512], start=True, stop=True)
                pt = ptp.tile([128, 512], BF16, tag='pt', name=f'pt{lc}_{jt}')
                nc.scalar.activation(pt[:], e[:], mybir.ActivationFunctionType.Exp,
                                     bias=0.0, scale=SCALE)
                pts.append(pt)

            def emit_av(lc, jt, pts):
                nc.tensor.matmul(o65[lc][:], vt65[:, jt * 65:(jt + 1) * 65], pts[jt][:],
                                 start=(jt == 0), stop=(jt == NJT - 1))

            # head of C(0)
            pts_by_lc = {0: []}
            for jt in range(HOIST):
                emit_cunit(0, jt, pts_by_lc[0])
            for lc in range(NLC):
                # software-pipelined across the lc boundary: the first HOIST
                # E^T+exp units of lc were emitted before the previous chunk's
                # stats batch, so ACT has queued work while DVE chews stats.
                pts = pts_by_lc[lc]
                for jt in range(HOIST, NJT):
                    emit_cunit(lc, jt, pts)
                    emit_av(lc, jt - HOIST, pts)
                for jt in range(NJT - HOIST, NJT):
                    emit_av(lc, jt, pts)
                if lc + 1 < NLC:
                    emit_brow(lc + 1)
                    pts_by_lc[lc + 1] = []
                    for jt in range(HOIST):
                        emit_cunit(lc + 1, jt, pts_by_lc[lc + 1])
                if lc + 2 < NLC:
                    emit_stats(lc + 2)
                emit_norm(lc)


    nc.finalize()
    return nc


_cache = {}


def _get_nc(gamma: float):
    key = float(gamma)
    if key not in _cache:
        _cache[key] = build_nc(key)
    return _cache[key]


def _in_maps(inputs):
    x = np.asarray(inputs['x'], np.float32)
    vwT = f32r_round(np.asarray(inputs['value_w'], np.float32).T)
    qwT = f32r_round(np.asarray(inputs['query_w'], np.float32).T / SCALE)
    kwT = f32r_round(np.asarray(inputs['key_w'], np.float32).T)
    ident = np.eye(128, dtype=np.float32)
    xs = f32r_round(x[..., 0])
    return [
        {'x': np.ascontiguousarray(xs[b]), 'vwT': vwT, 'qwT': qwT, 'kwT': kwT, 'ident': ident}
        for b in range(B)
    ]


def kernel(x, value_w, value_b, query_w, query_b, key_w, key_b, gamma):
    gamma_f = float(np.asarray(gamma).reshape(-1)[0])
    nc = _get_nc(gamma_f)
    maps = _in_maps(dict(x=x, value_w=value_w, query_w=query_w, key_w=key_w))
    res = run_bass_kernel_spmd(nc, maps, core_ids=list(range(B)), trace=False)
    out = np.stack([res.results[b]['out'] for b in range(B)], axis=0)
    return out[..., None].astype(np.float32)


def run_traced(inputs):
    gamma_f = float(np.asarray(inputs['gamma']).reshape(-1)[0])
    nc = _get_nc(gamma_f)
    maps = _in_maps(inputs)
    res = run_bass_kernel_spmd(nc, maps, core_ids=list(range(B)), trace=True)
    out = np.stack([res.results[b]['out'] for b in range(B)], axis=0)
    return out[..., None].astype(np.float32), res.exec_time_ns

